# revision 1
# baseline (speedup 1.0000x reference)
"""AUGRU cell (attention-gated GRU update) on 8 Trainium2 NeuronCores.

Data-parallel: the batch dim (16384) of x / att_score / hidden is sharded
across 8 cores (2048 rows each); the six 512x512 weight matrices are
replicated.

Per-core dataflow (per 128-row batch tile, 16 tiles):
  zu = x @ W_u + h @ U_u          (PSUM accum, 8 matmuls, f32r fast path)
  zr = x @ W_r + h @ U_r
  xh = x @ W_h ; hu = h @ U_h
  u = att * sigmoid(zu); r = sigmoid(zr)
  hhat = tanh(xh + r * hu)
  out = h + u * (hhat - h)

Matmuls run in float32r by default (full-rate fp32 PE mode at N>=256,
tf32-class rounding, output rel err ~1.5e-4); MM_DTYPE=bf16 selects bf16
matmuls (~10% faster, rel err ~2e-3). Matmul operands are staged through
DMA + DVE tensor_copy into resident SBUF tiles and each PSUM bank has a
single releasing engine, which keeps per-Matmult sync waits at <=1; any
instruction that still ends up with more waits is legalized by
_split_multi_waits (this walrus accepts only one sync wait per
instruction).
"""

import os
import sys

if "/opt/trn_rl_repo" not in sys.path:
    sys.path.insert(0, "/opt/trn_rl_repo")

import numpy as np

NCORES = 8
P = 128
MM_DTYPE = os.environ.get("MM_DTYPE", "f32r")  # "f32r" (tf32-class, rel ~1.5e-4) or "bf16" (rel ~2e-3, faster LDW)

_PROGRAM_CACHE = {}


def _split_multi_waits(nc):
    """walrus codegen accepts at most ONE sync wait per instruction (the
    TPB EVENTS struct has a single wait slot and setupSyncWait refuses to
    spill).  Tile's add_semaphores can emit several waits on one
    instruction; hoist all but the last into same-engine no-ops inserted
    immediately before it.  The engine executes the no-ops (each blocking
    on one semaphore) then the instruction — identical semantics."""
    import concourse.mybir as mybir

    for fn in nc.m.functions:
        for blk in fn.blocks:
            insts = blk.instructions
            i = 0
            while i < len(insts):
                inst = insts[i]
                si = inst.sync_info
                if si is not None and len(si.on_wait) > 1:
                    waits = list(si.on_wait)
                    inst.sync_info = mybir.SyncInfo(
                        on_wait=waits[-1:], on_update=list(si.on_update)
                    )
                    for j, w in enumerate(waits[:-1]):
                        nop = mybir.InstNoOp(
                            name=nc.get_next_instruction_name(),
                            sync_info=mybir.SyncInfo(on_wait=[w], on_update=[]),
                            bass_nofuse=True,
                            engine=inst.engine,
                        )
                        nc.register_instruction(nop)
                        insts.insert(i + j, nop)
                    i += len(waits) - 1
                i += 1


def _build_program(D, H, Bc, with_bias, mm_dtype=None):
    import concourse.bass as bass
    import concourse.mybir as mybir
    import concourse.tile as tile
    from concourse.alu_op_type import AluOpType

    f32 = mybir.dt.float32
    bf16_mode = (mm_dtype or MM_DTYPE) == "bf16"
    f32r = mybir.dt.bfloat16 if bf16_mode else mybir.dt.float32r
    stg_dt = mybir.dt.bfloat16 if bf16_mode else f32
    Sig = mybir.ActivationFunctionType.Sigmoid
    Tanh = mybir.ActivationFunctionType.Tanh

    KD = D // P  # K chunks for x-side matmuls
    KH = H // P  # K chunks for h-side matmuls
    TILES = Bc // P
    BCH = 256  # batch-axis chunk for staging the big transposed loads
    NB = Bc // BCH

    nc = bass.Bass()
    xT_p = nc.declare_dram_parameter("xT", [D, Bc], stg_dt, isOutput=False)
    hT_p = nc.declare_dram_parameter("hT", [H, Bc], stg_dt, isOutput=False)
    hN_p = nc.declare_dram_parameter("hN", [Bc, H], f32, isOutput=False)
    att_p = nc.declare_dram_parameter("att", [P, TILES], f32, isOutput=False)
    wnames = ("wu", "wr", "wh", "uu", "ur", "uh")
    w_p = {n: nc.declare_dram_parameter(n, [D if n[0] == "w" else H, H], stg_dt,
                                        isOutput=False) for n in wnames}
    if with_bias:
        b_p = {n: nc.declare_dram_parameter(n, [P, H], f32, isOutput=False)
               for n in ("bub", "brb", "bhb")}
    out_p = nc.declare_dram_parameter("out", [Bc, H], f32, isOutput=True)

    with tile.TileContext(nc) as tc:
        with (
            tc.tile_pool(name="w", bufs=1) as wpool,
            tc.tile_pool(name="stg", bufs=4) as spool,
            tc.tile_pool(name="stgw", bufs=8) as swpool,
            tc.tile_pool(name="dat", bufs=4) as dpool,
            tc.tile_pool(name="ep", bufs=3) as epool,
            tc.tile_pool(name="ps", bufs=2, space="PSUM") as ppool,
        ):
            # Resident matmul operands, DMA'd to staging then DVE-copied
            # into f32r/bf16 tiles: a single producing engine for all PE
            # operands keeps matmul wait counts at <=1 (fewer PE-queue
            # no-ops from _split_multi_waits, measurably faster than
            # DMA-direct). Chunked copies give fine-grained deps.
            w_sb = {n: wpool.tile([P, KD if n[0] == "w" else KH, H], f32r,
                                  tag=n, name=f"w_{n}") for n in wnames}
            wviews = {n: w_p[n][:].rearrange("(ko ki) n -> ki ko n", ki=P)
                      for n in wnames}
            xT_sb = wpool.tile([P, KD, Bc], f32r, tag="xT")
            hT_sb = wpool.tile([P, KH, Bc], f32r, tag="hT")
            xview = xT_p[:].rearrange("(ko ki) b -> ki ko b", ki=P)
            hview = hT_p[:].rearrange("(ko ki) b -> ki ko b", ki=P)

            def stage_chunk(lo, size, h_eng=None):
                bs = slice(lo, lo + size)
                stg = spool.tile([P, KD, BCH], stg_dt, tag="xs")
                nc.sync.dma_start(stg[:, :, :size], xview[:, :, bs])
                nc.vector.tensor_copy(xT_sb[:, :, bs], stg[:, :, :size])
                stg = spool.tile([P, KH, BCH], stg_dt, tag="hs")
                (h_eng or nc.sync).dma_start(stg[:, :, :size], hview[:, :, bs])
                nc.vector.tensor_copy(hT_sb[:, :, bs], stg[:, :, :size])

            def stage_weight(n, ko, eng):
                # h-side weights ride the ACT HWDGE queue (idle until the
                # first epilogue ~20us in) so their issue overlaps SP's:
                # one queue's ~0.7us per-dma_start cost otherwise keeps
                # the PE waiting for U_* through the first few tiles.
                stg = swpool.tile([P, H], stg_dt, tag="ws")
                eng.dma_start(stg, wviews[n][:, ko])
                nc.vector.tensor_copy(w_sb[n][:, ko], stg)

            # PE warm-up: the HAM clock gate needs ~3.4us of sustained
            # PE activity before it lifts the array clock from 1.2 to
            # 2.4 GHz. Junk bf16 weight loads (legal standalone, unlike
            # f32r) keep the PE busy while the first DMAs land, so the
            # real matmuls start warm.
            warm = wpool.tile([P, P], mybir.dt.bfloat16, tag="warm")
            nc.vector.memset(warm, 0.0)
            for _ in range(32):
                nc.tensor.ldweights(warm)

            # Order the preamble by first consumption: a small first
            # batch chunk + x-side weights, then h-side weights and the
            # remaining batch chunks.
            # First h-chunk DMA rides the ACT queue: h-side data isn't
            # consumed until matmul #13, and this moves every x-side
            # weight DMA one SP issue slot (~0.7us) earlier.
            stage_chunk(0, P, h_eng=nc.scalar)
            for ko in range(KD):
                for n in ("wh", "wu", "wr"):
                    stage_weight(n, ko, nc.sync)
            for ko in range(KH):
                for n in ("uh", "uu", "ur"):
                    stage_weight(n, ko, nc.scalar)
            stage_chunk(P, P)
            for c in range(1, NB):
                stage_chunk(c * BCH, BCH)
            w_sb = {(n, ko): w_sb[n][:, ko] for n in wnames
                    for ko in range(KD if n[0] == "w" else KH)}

            att_sb = wpool.tile([P, TILES], f32, tag="att")
            nc.sync.dma_start(att_sb, att_p[:])
            if with_bias:
                b_sb = {}
                for n in ("bub", "brb", "bhb"):
                    t = wpool.tile([P, H], f32, tag=n)
                    nc.sync.dma_start(t, b_p[n][:])
                    b_sb[n] = t

            for t in range(TILES):
                bsl = slice(t * P, (t + 1) * P)
                h_t = dpool.tile([P, H], f32, tag="h")
                nc.sync.dma_start(h_t, hN_p[bsl, :])

                p_zu = ppool.tile([P, H], f32, tag="zu")
                p_zr = ppool.tile([P, H], f32, tag="zr")
                p_xh = ppool.tile([P, H], f32, tag="xh")
                p_hh = ppool.tile([P, H], f32, tag="hh")

                # p_xh group first: the first matmul of a tile may need a
                # fresh xT chunk (DVE tick), and p_xh's PSUM slot is also
                # DVE-released, so its waits merge into one.
                for ki in range(KD):
                    lx = xT_sb[:, ki, bsl]
                    st = ki == 0
                    nc.tensor.matmul(p_xh, lx, w_sb["wh", ki],
                                     start=st, stop=ki == KD - 1)
                    nc.tensor.matmul(p_zu, lx, w_sb["wu", ki], start=st, stop=False)
                    nc.tensor.matmul(p_zr, lx, w_sb["wr", ki], start=st, stop=False)
                for ki in range(KH):
                    lh = hT_sb[:, ki, bsl]
                    last = ki == KH - 1
                    nc.tensor.matmul(p_hh, lh, w_sb["uh", ki],
                                     start=ki == 0, stop=last)
                    nc.tensor.matmul(p_zu, lh, w_sb["uu", ki],
                                     start=False, stop=last)
                    nc.tensor.matmul(p_zr, lh, w_sb["ur", ki],
                                     start=False, stop=last)

                # Epilogue. Each PSUM bank has exactly one releasing engine
                # (zu/zr: ACT sigmoid; xh/hh: DVE), keeping every matmul's
                # wait count at <=1.
                u = epool.tile([P, H], f32, tag="u")
                r = epool.tile([P, H], f32, tag="r")
                g = epool.tile([P, H], f32, tag="g")
                o = epool.tile([P, H], f32, tag="o")

                if with_bias:
                    zu_s = epool.tile([P, H], f32, tag="zu_s")
                    zr_s = epool.tile([P, H], f32, tag="zr_s")
                    nc.vector.tensor_add(zu_s, p_zu, b_sb["bub"])
                    nc.vector.tensor_add(zr_s, p_zr, b_sb["brb"])
                    nc.scalar.activation(u, zu_s, Sig)
                    nc.scalar.activation(r, zr_s, Sig)
                else:
                    nc.scalar.activation(u, p_zu, Sig)
                    nc.scalar.activation(r, p_zr, Sig)
                nc.vector.tensor_mul(g, r, p_hh)       # r * (h @ U_h)
                nc.vector.tensor_add(g, g, p_xh)       # + x @ W_h
                if with_bias:
                    nc.vector.tensor_add(g, g, b_sb["bhb"])
                nc.scalar.activation(g, g, Tanh)       # hhat
                nc.vector.tensor_sub(g, g, h_t)        # hhat - h
                # g = (g * att) * u  == att*sigmoid(zu) * (hhat - h)
                nc.vector.scalar_tensor_tensor(
                    g, g, att_sb[:, t:t + 1], u, AluOpType.mult, AluOpType.mult
                )
                nc.vector.tensor_add(o, g, h_t)        # h + u*(hhat - h)
                nc.sync.dma_start(out_p[bsl, :], o)

    _split_multi_waits(nc)
    return nc


def check_waits(nc):
    """Matmults and Drains may carry at most 1 sync wait on walrus; other
    instruction classes tolerate more (walrus splits them itself)."""
    bad = []
    for fn in nc.m.functions:
        for blk in fn.blocks:
            for inst in blk.instructions:
                si = inst.sync_info
                nw = len(si.on_wait) if si else 0
                kind = type(inst).__name__
                if nw > 1:
                    bad.append((inst.name, kind, nw))
    return bad


def _get_program(D, H, Bc, with_bias):
    key = (D, H, Bc, with_bias, MM_DTYPE)
    if key not in _PROGRAM_CACHE:
        nc = _build_program(D, H, Bc, with_bias)
        bad = check_waits(nc)
        if bad:
            raise RuntimeError(f"instructions over the sync-wait limit: {bad}")
        _PROGRAM_CACHE[key] = nc
    return _PROGRAM_CACHE[key]


def _np32(a):
    return np.ascontiguousarray(np.asarray(a, dtype=np.float32))


def _mm_cast(a):
    if MM_DTYPE == "bf16":
        import ml_dtypes

        return np.ascontiguousarray(a.astype(ml_dtypes.bfloat16))
    return a


def _prepare(x, att_score, hidden, W_u, U_u, b_u, W_r, U_r, b_r, W_h, U_h, b_h):
    x = _np32(x)
    att_score = _np32(att_score)
    hidden = _np32(hidden)
    B, D = x.shape
    H = hidden.shape[1]
    assert B % (NCORES * P) == 0 and D % P == 0 and H % P == 0
    Bc = B // NCORES

    weights = {
        "wu": _np32(W_u), "wr": _np32(W_r), "wh": _np32(W_h),
        "uu": _np32(U_u), "ur": _np32(U_r), "uh": _np32(U_h),
    }
    biases = [_np32(b_u), _np32(b_r), _np32(b_h)]
    with_bias = any(np.any(b) for b in biases)
    cast_weights = {k: _mm_cast(v) for k, v in weights.items()}

    in_maps = []
    for c in range(NCORES):
        sl = slice(c * Bc, (c + 1) * Bc)
        xs, hs, at = x[sl], hidden[sl], att_score[sl]
        m = {
            "xT": _mm_cast(np.ascontiguousarray(xs.T)),
            "hT": _mm_cast(np.ascontiguousarray(hs.T)),
            "hN": np.ascontiguousarray(hs),
            "att": np.ascontiguousarray(at.reshape(Bc // P, P).T),
        }
        m.update(cast_weights)
        if with_bias:
            m["bub"] = np.ascontiguousarray(np.broadcast_to(biases[0], (P, H)))
            m["brb"] = np.ascontiguousarray(np.broadcast_to(biases[1], (P, H)))
            m["bhb"] = np.ascontiguousarray(np.broadcast_to(biases[2], (P, H)))
        in_maps.append(m)

    nc = _get_program(D, H, Bc, with_bias)
    return nc, in_maps


def _run(inputs, trace=False, **trace_kwargs):
    from concourse.bass_utils import run_bass_kernel_spmd

    nc, in_maps = _prepare(**inputs)
    res = run_bass_kernel_spmd(nc, in_maps, list(range(NCORES)), trace=trace,
                               **trace_kwargs)
    out = np.concatenate([res.results[i]["out"] for i in range(NCORES)], axis=0)
    return out, res


def kernel(**inputs):
    out, _ = _run(inputs, trace=False)
    return out



# revision 3
# speedup vs baseline: 1.1561x; 1.1561x over previous
"""AUGRU cell (attention-gated GRU update) on 8 Trainium2 NeuronCores.

Data-parallel: the batch dim (16384) of x / att_score / hidden is sharded
across 8 cores (2048 rows each); the six 512x512 weight matrices are
replicated.

Per-core dataflow (per 128-row batch tile, 16 tiles):
  zu = x @ W_u + h @ U_u          (PSUM accum, 8 matmuls)
  zr = x @ W_r + h @ U_r
  hu = h @ U_h ; xh = x @ W_h     (candidate path last: shortest tail)
  u2 = att * sigmoid(zu); r = sigmoid(zr)
  hhat = tanh(r * hu + xh)
  out = (1 - u2) * h + u2 * hhat

Matmuls in bf16 (rel err ~2.4e-3 vs the 2e-2 gate). All matmul operands
are HOST-PREPACKED into DRAM buffers whose per-partition bytes are
contiguous in exactly the SBUF-resident layout, so every load is one
direct DMA (128 descriptors x 2-4KB) - no staging copies, no DVE casts.
DMAs are spread across the sync/scalar/gpsimd/vector engine queues so
weight and batch-chunk loads issue in parallel right after the framework
preamble; junk bf16 ldweights keep the PE busy (HAM warm) while the
first weights land. Each PSUM bank keeps a single releasing engine
(zu/zr: ACT sigmoid; hu/xh: DVE) so per-Matmult sync waits stay <=1;
stragglers are legalized by _split_multi_waits.
"""

import os
import sys

if "/opt/trn_rl_repo" not in sys.path:
    sys.path.insert(0, "/opt/trn_rl_repo")

import numpy as np

NCORES = 8
P = 128
CHB = 512  # batch-axis chunk width for x/h loads
MM_DTYPE = os.environ.get("MM_DTYPE", "bf16")  # "bf16" or "f32r"

_PROGRAM_CACHE = {}


def _split_multi_waits(nc):
    """walrus codegen accepts at most ONE sync wait per instruction (the
    TPB EVENTS struct has a single wait slot and setupSyncWait refuses to
    spill).  Tile's add_semaphores can emit several waits on one
    instruction; hoist all but the last into same-engine no-ops inserted
    immediately before it.  The engine executes the no-ops (each blocking
    on one semaphore) then the instruction - identical semantics."""
    import concourse.mybir as mybir

    for fn in nc.m.functions:
        for blk in fn.blocks:
            insts = blk.instructions
            i = 0
            while i < len(insts):
                inst = insts[i]
                si = inst.sync_info
                if si is not None and len(si.on_wait) > 1:
                    waits = list(si.on_wait)
                    inst.sync_info = mybir.SyncInfo(
                        on_wait=waits[-1:], on_update=list(si.on_update)
                    )
                    for j, w in enumerate(waits[:-1]):
                        nop = mybir.InstNoOp(
                            name=nc.get_next_instruction_name(),
                            sync_info=mybir.SyncInfo(on_wait=[w], on_update=[]),
                            bass_nofuse=True,
                            engine=inst.engine,
                        )
                        nc.register_instruction(nop)
                        insts.insert(i + j, nop)
                    i += len(waits) - 1
                i += 1


def _build_program(D, H, Bc, with_bias, mm_dtype=None):
    import concourse.bass as bass
    import concourse.mybir as mybir
    import concourse.tile as tile
    from concourse.alu_op_type import AluOpType

    f32 = mybir.dt.float32
    bf16_mode = (mm_dtype or MM_DTYPE) == "bf16"
    mm_dt = mybir.dt.bfloat16 if bf16_mode else mybir.dt.float32r
    Sig = mybir.ActivationFunctionType.Sigmoid
    Tanh = mybir.ActivationFunctionType.Tanh

    KD = D // P  # K chunks for x-side matmuls
    KH = H // P  # K chunks for h-side matmuls
    TILES = Bc // P
    NCH = Bc // CHB  # batch chunks for the x/h loads

    nc = bass.Bass()
    # Host-prepacked DRAM layouts: per-partition bytes contiguous, matching
    # the SBUF-resident tiles exactly (one fat descriptor per partition).
    xT_p = nc.declare_dram_parameter("xT", [P, NCH * KD * CHB], mm_dt, isOutput=False)
    hT_p = nc.declare_dram_parameter("hT", [P, NCH * KH * CHB], mm_dt, isOutput=False)
    hN_p = nc.declare_dram_parameter("hN", [Bc, H], f32, isOutput=False)
    att_p = nc.declare_dram_parameter("att", [P, TILES], f32, isOutput=False)
    wnames = ("wu", "wr", "wh", "uu", "ur", "uh")
    w_p = {n: nc.declare_dram_parameter(n, [P, (KD if n[0] == "w" else KH) * H],
                                        mm_dt, isOutput=False) for n in wnames}
    if with_bias:
        b_p = {n: nc.declare_dram_parameter(n, [P, H], f32, isOutput=False)
               for n in ("bub", "brb", "bhb")}
    out_p = nc.declare_dram_parameter("out", [Bc, H], f32, isOutput=True)

    xview = xT_p[:].rearrange("ki (c ko b) -> ki c ko b", c=NCH, ko=KD)
    hview = hT_p[:].rearrange("ki (c ko b) -> ki c ko b", c=NCH, ko=KH)
    wview = {n: w_p[n][:].rearrange("ki (ko h) -> ki ko h",
                                    ko=KD if n[0] == "w" else KH) for n in wnames}

    with tile.TileContext(nc) as tc:
        with (
            tc.tile_pool(name="w", bufs=1) as wpool,
            tc.tile_pool(name="dat", bufs=4) as dpool,
            tc.tile_pool(name="ep", bufs=3) as epool,
            tc.tile_pool(name="ps", bufs=2, space="PSUM") as ppool,
        ):
            w_sb = {n: wpool.tile([P, KD if n[0] == "w" else KH, H], mm_dt,
                                  tag=n, name=f"w_{n}") for n in wnames}
            xT_sb = wpool.tile([P, KD, Bc], mm_dt, tag="xT")
            hT_sb = wpool.tile([P, KH, Bc], mm_dt, tag="hT")
            att_sb = wpool.tile([P, TILES], f32, tag="att")

            # PE warm-up: the HAM clock gate needs ~3.4us of sustained PE
            # activity before it lifts the array clock to 2.4 GHz. Junk
            # bf16 weight loads keep the PE busy while the first DMAs
            # land, so the real matmuls start warm. memset on gpsimd so
            # the LDWs are not gated behind any DMA-issuing engine.
            warm = wpool.tile([P, P], mybir.dt.bfloat16, tag="warm")
            nc.gpsimd.memset(warm, 0.0)
            for _ in range(24):
                nc.tensor.ldweights(warm)

            # Direct DMAs, spread across the three HWDGE queues (sync,
            # scalar, gpsimd), in consumption order (per-tile matmul
            # groups run zu, zr, hu, xh):
            #   scalar: wu, uu, wh   then per-tile hN loads + activations
            #   sync:   wr, ur, att  then per-tile out stores
            #   gpsimd: x/h chunk0, uh, x/h chunks 1..  (+ epilogue ops)
            nc.scalar.dma_start(w_sb["wu"], wview["wu"])
            nc.sync.dma_start(w_sb["wr"], wview["wr"])
            nc.gpsimd.dma_start(xT_sb[:, :, 0:CHB], xview[:, 0])
            nc.gpsimd.dma_start(hT_sb[:, :, 0:CHB], hview[:, 0])
            nc.scalar.dma_start(w_sb["uu"], wview["uu"])
            nc.sync.dma_start(w_sb["ur"], wview["ur"])
            nc.gpsimd.dma_start(w_sb["uh"], wview["uh"])
            nc.scalar.dma_start(w_sb["wh"], wview["wh"])
            nc.sync.dma_start(att_sb, att_p[:])
            for c in range(1, NCH):
                nc.gpsimd.dma_start(xT_sb[:, :, c * CHB:(c + 1) * CHB], xview[:, c])
                nc.gpsimd.dma_start(hT_sb[:, :, c * CHB:(c + 1) * CHB], hview[:, c])
            if with_bias:
                b_sb = {}
                for n in ("bub", "brb", "bhb"):
                    t = wpool.tile([P, H], f32, tag=n)
                    nc.sync.dma_start(t, b_p[n][:])
                    b_sb[n] = t

            for t in range(TILES):
                bsl = slice(t * P, (t + 1) * P)
                h_t = dpool.tile([P, H], f32, tag="h")
                nc.scalar.dma_start(h_t, hN_p[bsl, :])

                p_zu = ppool.tile([P, H], f32, tag="zu")
                p_zr = ppool.tile([P, H], f32, tag="zr")
                p_hu = ppool.tile([P, H], f32, tag="hu")
                p_xh = ppool.tile([P, H], f32, tag="xh")

                lx = [xT_sb[:, ki, bsl] for ki in range(KD)]
                lh = [hT_sb[:, ki, bsl] for ki in range(KH)]

                # Group order zu, zr, hu, xh: the gates finish early (their
                # epilogue work overlaps the candidate matmuls), the
                # candidate path finishes last so the post-last-matmul
                # chain is just add/tanh/mul/add/store.
                for ki in range(KD):
                    nc.tensor.matmul(p_zu, lx[ki], w_sb["wu"][:, ki],
                                     start=ki == 0, stop=False)
                for ki in range(KH):
                    nc.tensor.matmul(p_zu, lh[ki], w_sb["uu"][:, ki],
                                     start=False, stop=ki == KH - 1)
                for ki in range(KD):
                    nc.tensor.matmul(p_zr, lx[ki], w_sb["wr"][:, ki],
                                     start=ki == 0, stop=False)
                for ki in range(KH):
                    nc.tensor.matmul(p_zr, lh[ki], w_sb["ur"][:, ki],
                                     start=False, stop=ki == KH - 1)
                for ki in range(KH):
                    nc.tensor.matmul(p_hu, lh[ki], w_sb["uh"][:, ki],
                                     start=ki == 0, stop=ki == KH - 1)
                for ki in range(KD):
                    nc.tensor.matmul(p_xh, lx[ki], w_sb["wh"][:, ki],
                                     start=ki == 0, stop=ki == KD - 1)

                # Epilogue. PSUM releasing engines: zu/zr by ACT sigmoid,
                # hu/xh by DVE. The last tile runs the epilogue in two
                # H-halves so the tail chain after the final matmul is
                # half-length.
                u = epool.tile([P, H], f32, tag="u")
                r = epool.tile([P, H], f32, tag="r")
                u2 = epool.tile([P, H], f32, tag="u2")
                hm = epool.tile([P, H], f32, tag="hm")
                g = epool.tile([P, H], f32, tag="g")
                o = epool.tile([P, H], f32, tag="o")
                if with_bias:
                    zus = epool.tile([P, H], f32, tag="zus")
                    zrs = epool.tile([P, H], f32, tag="zrs")
                att_c = att_sb[:, t:t + 1]

                halves = ((slice(0, H),) if t < TILES - 1
                          else (slice(0, H // 2), slice(H // 2, H)))
                for hs in halves:
                    if with_bias:
                        nc.vector.tensor_add(zus[:, hs], p_zu[:, hs], b_sb["bub"][:, hs])
                        nc.vector.tensor_add(zrs[:, hs], p_zr[:, hs], b_sb["brb"][:, hs])
                        nc.scalar.activation(u[:, hs], zus[:, hs], Sig)
                        nc.scalar.activation(r[:, hs], zrs[:, hs], Sig)
                    else:
                        nc.scalar.activation(u[:, hs], p_zu[:, hs], Sig)
                        nc.scalar.activation(r[:, hs], p_zr[:, hs], Sig)
                    nc.vector.tensor_scalar_mul(u2[:, hs], u[:, hs], att_c)
                    # hm = (1 - u2) * h, off the critical chain on gpsimd
                    nc.gpsimd.tensor_scalar(hm[:, hs], u2[:, hs], -1.0, 1.0,
                                            AluOpType.mult, AluOpType.add)
                    nc.gpsimd.tensor_mul(hm[:, hs], hm[:, hs], h_t[:, hs])
                    nc.vector.tensor_mul(g[:, hs], r[:, hs], p_hu[:, hs])
                    nc.vector.tensor_add(g[:, hs], g[:, hs], p_xh[:, hs])
                    if with_bias:
                        nc.vector.tensor_add(g[:, hs], g[:, hs], b_sb["bhb"][:, hs])
                    nc.scalar.activation(g[:, hs], g[:, hs], Tanh)
                    nc.vector.tensor_mul(g[:, hs], g[:, hs], u2[:, hs])
                    nc.vector.tensor_add(o[:, hs], g[:, hs], hm[:, hs])
                    nc.sync.dma_start(out_p[bsl, hs], o[:, hs])

    _split_multi_waits(nc)
    return nc


def check_waits(nc):
    """Matmults and Drains may carry at most 1 sync wait on walrus; other
    instruction classes tolerate more (walrus splits them itself)."""
    bad = []
    for fn in nc.m.functions:
        for blk in fn.blocks:
            for inst in blk.instructions:
                si = inst.sync_info
                nw = len(si.on_wait) if si else 0
                kind = type(inst).__name__
                if nw > 1:
                    bad.append((inst.name, kind, nw))
    return bad


def _get_program(D, H, Bc, with_bias):
    key = (D, H, Bc, with_bias, MM_DTYPE)
    if key not in _PROGRAM_CACHE:
        nc = _build_program(D, H, Bc, with_bias)
        bad = check_waits(nc)
        if bad:
            raise RuntimeError(f"instructions over the sync-wait limit: {bad}")
        _PROGRAM_CACHE[key] = nc
    return _PROGRAM_CACHE[key]


def _np32(a):
    return np.ascontiguousarray(np.asarray(a, dtype=np.float32))


def _mm_np_dtype():
    if MM_DTYPE == "bf16":
        import ml_dtypes

        return ml_dtypes.bfloat16
    return np.float32


def _pack_bT(a, Bc, K, mmdt):
    """[Bc, K] activations -> [128, NCH*KO*CHB] with per-partition layout
    [chunk, ko, b_local], the exact SBUF-resident order (contiguous DMA)."""
    KO = K // P
    out = a.reshape(Bc // CHB, CHB, KO, P).transpose(3, 0, 2, 1)
    return np.ascontiguousarray(out.reshape(P, -1).astype(mmdt))


def _pack_w(w, mmdt):
    """[K, H] weight -> [128, KO*H] with per-partition layout [ko, h]."""
    K, H = w.shape
    out = w.reshape(K // P, P, H).transpose(1, 0, 2)
    return np.ascontiguousarray(out.reshape(P, -1).astype(mmdt))


def _prepare(x, att_score, hidden, W_u, U_u, b_u, W_r, U_r, b_r, W_h, U_h, b_h):
    x = _np32(x)
    att_score = _np32(att_score)
    hidden = _np32(hidden)
    B, D = x.shape
    H = hidden.shape[1]
    assert B % (NCORES * P) == 0 and D % P == 0 and H % P == 0
    Bc = B // NCORES
    mmdt = _mm_np_dtype()

    weights = {
        "wu": _np32(W_u), "wr": _np32(W_r), "wh": _np32(W_h),
        "uu": _np32(U_u), "ur": _np32(U_r), "uh": _np32(U_h),
    }
    biases = [_np32(b_u), _np32(b_r), _np32(b_h)]
    with_bias = any(np.any(b) for b in biases)
    packed_w = {k: _pack_w(v, mmdt) for k, v in weights.items()}

    in_maps = []
    for c in range(NCORES):
        sl = slice(c * Bc, (c + 1) * Bc)
        xs, hs, at = x[sl], hidden[sl], att_score[sl]
        m = {
            "xT": _pack_bT(xs, Bc, D, mmdt),
            "hT": _pack_bT(hs, Bc, H, mmdt),
            "hN": np.ascontiguousarray(hs),
            "att": np.ascontiguousarray(at.reshape(Bc // P, P).T),
        }
        m.update(packed_w)
        if with_bias:
            m["bub"] = np.ascontiguousarray(np.broadcast_to(biases[0], (P, H)))
            m["brb"] = np.ascontiguousarray(np.broadcast_to(biases[1], (P, H)))
            m["bhb"] = np.ascontiguousarray(np.broadcast_to(biases[2], (P, H)))
        in_maps.append(m)

    nc = _get_program(D, H, Bc, with_bias)
    return nc, in_maps


def _run(inputs, trace=False, **trace_kwargs):
    from concourse.bass_utils import run_bass_kernel_spmd

    nc, in_maps = _prepare(**inputs)
    res = run_bass_kernel_spmd(nc, in_maps, list(range(NCORES)), trace=trace,
                               **trace_kwargs)
    out = np.concatenate([res.results[i]["out"] for i in range(NCORES)], axis=0)
    return out, res


def kernel(**inputs):
    out, _ = _run(inputs, trace=False)
    return out


# revision 12
# speedup vs baseline: 1.1651x; 1.0078x over previous
"""AUGRU cell (attention-gated GRU update) on 8 Trainium2 NeuronCores.

Data-parallel: the batch dim (16384) of x / att_score / hidden is sharded
across 8 cores (2048 rows each); the six 512x512 weight matrices are
replicated.

Per-core dataflow (per 128-row batch tile, 16 tiles):
  zr = x @ W_r + h @ U_r          (PSUM accum, 8 matmuls)
  hu = h @ U_h ; xh = x @ W_h
  zu = x @ W_u + h @ U_u          (update gate last: shortest tail)
  r = sigmoid(zr); g = tanh(r * hu + xh); d = g - h
  u2 = att * sigmoid(zu)
  out = h + u2 * d                (== (1-u2)*h + u2*g)

Group order zr, hu, xh, zu means everything except the short
sigmoid(zu) -> u2 -> u2*d -> +h chain completes while the zu matmuls
still run; the last tile runs that chain in two H-halves to pipeline
ACT/DVE and cut the kernel tail.

Matmuls in bf16 (rel err ~2.4e-3 vs the 2e-2 gate). All matmul operands
are HOST-PREPACKED into DRAM buffers whose per-partition bytes are
contiguous in exactly the SBUF-resident layout, so every load is one
direct DMA (128 descriptors x 2-4KB) - no staging copies, no DVE casts.
DMAs are spread across the sync/scalar/gpsimd/vector engine queues so
weight and batch-chunk loads issue in parallel right after the framework
preamble; junk bf16 ldweights keep the PE busy (HAM warm) while the
first weights land. Each PSUM bank keeps a single releasing engine
(zu/zr: ACT sigmoid; hu/xh: DVE) so per-Matmult sync waits stay <=1;
stragglers are legalized by _split_multi_waits.
"""

import os
import sys

if "/opt/trn_rl_repo" not in sys.path:
    sys.path.insert(0, "/opt/trn_rl_repo")

import numpy as np

NCORES = 8
P = 128
MM_DTYPE = os.environ.get("MM_DTYPE", "bf16")  # "bf16" or "f32r"


def _bchunks(Bc):
    """Batch-chunk widths for the x/h loads: small early chunks so the
    first tiles' operands land fast, then wide chunks for bandwidth."""
    ws, rem = [], Bc
    for w in (P, P, 2 * P, 4 * P):
        if rem <= 0:
            break
        w = min(w, rem)
        ws.append(w)
        rem -= w
    while rem > 0:
        w = min(4 * P, rem)
        ws.append(w)
        rem -= w
    return ws

_PROGRAM_CACHE = {}


def _split_multi_waits(nc):
    """walrus codegen accepts at most ONE sync wait per instruction (the
    TPB EVENTS struct has a single wait slot and setupSyncWait refuses to
    spill).  Tile's add_semaphores can emit several waits on one
    instruction; hoist all but the last into same-engine no-ops inserted
    immediately before it.  The engine executes the no-ops (each blocking
    on one semaphore) then the instruction - identical semantics.

    Matmult/Ldweights get ALL waits hoisted: a wait carried on a PE
    instruction breaks the fill/drain overlap with the previous matmul
    (~210ns per occurrence, once per tile); a NoOp carrying the wait
    dispatches while the previous matmul still streams, so the pipeline
    stays full."""
    import concourse.mybir as mybir

    for fn in nc.m.functions:
        for blk in fn.blocks:
            insts = blk.instructions
            i = 0
            while i < len(insts):
                inst = insts[i]
                si = inst.sync_info
                nhoist = 0
                if si is not None and si.on_wait:
                    if type(inst).__name__ in ("InstMatmult", "InstLdweights"):
                        nhoist = len(si.on_wait)
                    elif len(si.on_wait) > 1:
                        nhoist = len(si.on_wait) - 1
                if nhoist:
                    waits = list(si.on_wait)
                    inst.sync_info = mybir.SyncInfo(
                        on_wait=waits[nhoist:], on_update=list(si.on_update)
                    )
                    for j, w in enumerate(waits[:nhoist]):
                        nop = mybir.InstNoOp(
                            name=nc.get_next_instruction_name(),
                            sync_info=mybir.SyncInfo(on_wait=[w], on_update=[]),
                            bass_nofuse=True,
                            engine=inst.engine,
                        )
                        nc.register_instruction(nop)
                        insts.insert(i + j, nop)
                    i += nhoist
                i += 1


def _build_program(D, H, Bc, with_bias, mm_dtype=None):
    import concourse.bass as bass
    import concourse.mybir as mybir
    import concourse.tile as tile
    from concourse.alu_op_type import AluOpType

    f32 = mybir.dt.float32
    bf16_mode = (mm_dtype or MM_DTYPE) == "bf16"
    mm_dt = mybir.dt.bfloat16 if bf16_mode else mybir.dt.float32r
    Sig = mybir.ActivationFunctionType.Sigmoid
    Tanh = mybir.ActivationFunctionType.Tanh

    KD = D // P  # K chunks for x-side matmuls
    KH = H // P  # K chunks for h-side matmuls
    TILES = Bc // P

    nc = bass.Bass()
    # Host-prepacked DRAM layouts: per-partition bytes contiguous, matching
    # the SBUF-resident tiles exactly (one fat descriptor per partition).
    xT_p = nc.declare_dram_parameter("xT", [P, KD * Bc], mm_dt, isOutput=False)
    hT_p = nc.declare_dram_parameter("hT", [P, KH * Bc], mm_dt, isOutput=False)
    hN_p = nc.declare_dram_parameter("hN", [Bc, H], f32, isOutput=False)
    att_p = nc.declare_dram_parameter("att", [P, TILES], f32, isOutput=False)
    wnames = ("wu", "wr", "wh", "uu", "ur", "uh")
    w_p = {n: nc.declare_dram_parameter(n, [P, (KD if n[0] == "w" else KH) * H],
                                        mm_dt, isOutput=False) for n in wnames}
    if with_bias:
        b_p = {n: nc.declare_dram_parameter(n, [P, H], f32, isOutput=False)
               for n in ("bub", "brb", "bhb")}
    out_p = nc.declare_dram_parameter("out", [Bc, H], f32, isOutput=True)

    wview = {n: w_p[n][:].rearrange("ki (ko h) -> ki ko h",
                                    ko=KD if n[0] == "w" else KH) for n in wnames}

    CH_W = _bchunks(Bc)

    with tile.TileContext(nc) as tc:
        with (
            tc.tile_pool(name="w", bufs=1) as wpool,
            tc.tile_pool(name="dat", bufs=4) as dpool,
            tc.tile_pool(name="ep", bufs=3) as epool,
            tc.tile_pool(name="ps", bufs=2, space="PSUM") as ppool,
        ):
            w_sb = {n: wpool.tile([P, KD if n[0] == "w" else KH, H], mm_dt,
                                  tag=n, name=f"w_{n}") for n in wnames}
            xT_sb = wpool.tile([P, KD, Bc], mm_dt, tag="xT")
            hT_sb = wpool.tile([P, KH, Bc], mm_dt, tag="hT")
            att_sb = wpool.tile([P, TILES], f32, tag="att")

            # PE warm-up: the HAM clock gate needs ~3.4us of sustained PE
            # activity before it lifts the array clock to 2.4 GHz. Junk
            # bf16 weight loads keep the PE busy while the first DMAs
            # land, so the real matmuls start warm. memset on gpsimd so
            # the LDWs are not gated behind any DMA-issuing engine.
            warm = wpool.tile([P, P], mybir.dt.bfloat16, tag="warm")
            nc.gpsimd.memset(warm, 0.0)
            for _ in range(24):
                nc.tensor.ldweights(warm)

            # Direct DMAs, spread across the three HWDGE queues (sync,
            # scalar, gpsimd), in consumption order (per-tile matmul
            # groups run zr, hu, xh, zu -> weight need order is
            # wr, ur, uh, wh, wu, uu with ~1us spacing):
            #   sync:   x chunk0, wr halves, wu, att   then out stores
            #   scalar: ur, wh   then per-tile hN loads + activations
            #   gpsimd: h chunk0, uh, remaining x/h chunks, uu
            def chunk_dma(eng, sb, view, c):
                lo = sum(CH_W[:c])
                KO = sb.shape[1]
                src = view[:, KO * lo:KO * (lo + CH_W[c])].rearrange(
                    "ki (ko b) -> ki ko b", ko=KO)
                eng.dma_start(sb[:, :, lo:lo + CH_W[c]], src)

            nc.sync.dma_start(xT_sb[:, :, 0:CH_W[0]],
                              xT_p[:, 0:KD * CH_W[0]].rearrange(
                                  "ki (ko b) -> ki ko b", ko=KD))
            nc.gpsimd.dma_start(hT_sb[:, :, 0:CH_W[0]],
                                hT_p[:, 0:KH * CH_W[0]].rearrange(
                                    "ki (ko b) -> ki ko b", ko=KH))
            nc.sync.dma_start(w_sb["wr"][:, 0:KD // 2], wview["wr"][:, 0:KD // 2])
            nc.sync.dma_start(w_sb["wr"][:, KD // 2:], wview["wr"][:, KD // 2:])
            nc.scalar.dma_start(w_sb["ur"], wview["ur"])
            nc.gpsimd.dma_start(w_sb["uh"], wview["uh"])
            nc.scalar.dma_start(w_sb["wh"], wview["wh"])
            nc.sync.dma_start(w_sb["wu"], wview["wu"])
            nc.sync.dma_start(att_sb, att_p[:])
            for c in range(1, len(CH_W)):
                chunk_dma(nc.gpsimd, xT_sb, xT_p[:], c)
                chunk_dma(nc.gpsimd, hT_sb, hT_p[:], c)
                if c == 1:
                    nc.gpsimd.dma_start(w_sb["uu"], wview["uu"])
            if with_bias:
                b_sb = {}
                for n in ("bub", "brb", "bhb"):
                    t = wpool.tile([P, H], f32, tag=n)
                    nc.scalar.dma_start(t, b_p[n][:])
                    b_sb[n] = t

            for t in range(TILES):
                bsl = slice(t * P, (t + 1) * P)
                h_t = dpool.tile([P, H], f32, tag="h")
                nc.scalar.dma_start(h_t, hN_p[bsl, :])

                p_zr = ppool.tile([P, H], f32, tag="zr")
                p_hu = ppool.tile([P, H], f32, tag="hu")
                p_xh = ppool.tile([P, H], f32, tag="xh")
                p_zu = ppool.tile([P, H], f32, tag="zu")

                lx = [xT_sb[:, ki, bsl] for ki in range(KD)]
                lh = [hT_sb[:, ki, bsl] for ki in range(KH)]

                # Group order zr, hu, xh, zu: the whole candidate chain
                # (r, r*hu+xh, tanh, -h) completes while the zu matmuls
                # still run; only sig(zu) -> u2 -> u2*d -> +h trails the
                # last matmul.
                for ki in range(KD):
                    nc.tensor.matmul(p_zr, lx[ki], w_sb["wr"][:, ki],
                                     start=ki == 0, stop=False)
                for ki in range(KH):
                    nc.tensor.matmul(p_zr, lh[ki], w_sb["ur"][:, ki],
                                     start=False, stop=ki == KH - 1)
                for ki in range(KH):
                    nc.tensor.matmul(p_hu, lh[ki], w_sb["uh"][:, ki],
                                     start=ki == 0, stop=ki == KH - 1)
                for ki in range(KD):
                    nc.tensor.matmul(p_xh, lx[ki], w_sb["wh"][:, ki],
                                     start=ki == 0, stop=ki == KD - 1)
                for ki in range(KD):
                    nc.tensor.matmul(p_zu, lx[ki], w_sb["wu"][:, ki],
                                     start=ki == 0, stop=False)
                for ki in range(KH):
                    nc.tensor.matmul(p_zu, lh[ki], w_sb["uu"][:, ki],
                                     start=False, stop=ki == KH - 1)

                # Epilogue. PSUM releasing engines: zr/zu by ACT sigmoid,
                # hu/xh by DVE.
                u = epool.tile([P, H], f32, tag="u")
                r = epool.tile([P, H], f32, tag="r")
                u2 = epool.tile([P, H], f32, tag="u2")
                g = epool.tile([P, H], f32, tag="g")
                d = epool.tile([P, H], f32, tag="d")
                o = epool.tile([P, H], f32, tag="o")
                if with_bias:
                    zus = epool.tile([P, H], f32, tag="zus")
                    zrs = epool.tile([P, H], f32, tag="zrs")
                att_c = att_sb[:, t:t + 1]

                if with_bias:
                    nc.vector.tensor_add(zrs, p_zr, b_sb["brb"])
                    nc.scalar.activation(r, zrs, Sig)
                else:
                    nc.scalar.activation(r, p_zr, Sig)
                nc.vector.tensor_mul(g, r, p_hu)       # r * (h @ U_h)
                nc.vector.tensor_add(g, g, p_xh)       # + x @ W_h
                if with_bias:
                    nc.vector.tensor_add(g, g, b_sb["bhb"])
                nc.scalar.activation(g, g, Tanh)       # hhat
                nc.vector.tensor_sub(d, g, h_t)        # hhat - h

                # u2 = att * sigmoid(zu); out = h + u2 * d. Runs in two
                # H-halves on the last tile so ACT/DVE pipeline and the
                # final store issues earlier.
                halves = ((slice(0, H),) if t < TILES - 1
                          else (slice(0, H // 2), slice(H // 2, H)))
                for hs in halves:
                    if with_bias:
                        nc.vector.tensor_add(zus[:, hs], p_zu[:, hs], b_sb["bub"][:, hs])
                        nc.scalar.activation(u[:, hs], zus[:, hs], Sig)
                    else:
                        nc.scalar.activation(u[:, hs], p_zu[:, hs], Sig)
                    nc.vector.tensor_scalar_mul(u2[:, hs], u[:, hs], att_c)
                    nc.vector.tensor_mul(u2[:, hs], u2[:, hs], d[:, hs])
                    nc.vector.tensor_add(o[:, hs], u2[:, hs], h_t[:, hs])
                    nc.sync.dma_start(out_p[bsl, hs], o[:, hs])

    _split_multi_waits(nc)
    return nc


def check_waits(nc):
    """Matmults and Drains may carry at most 1 sync wait on walrus; other
    instruction classes tolerate more (walrus splits them itself)."""
    bad = []
    for fn in nc.m.functions:
        for blk in fn.blocks:
            for inst in blk.instructions:
                si = inst.sync_info
                nw = len(si.on_wait) if si else 0
                kind = type(inst).__name__
                if nw > 1:
                    bad.append((inst.name, kind, nw))
    return bad


def _get_program(D, H, Bc, with_bias):
    key = (D, H, Bc, with_bias, MM_DTYPE)
    if key not in _PROGRAM_CACHE:
        nc = _build_program(D, H, Bc, with_bias)
        bad = check_waits(nc)
        if bad:
            raise RuntimeError(f"instructions over the sync-wait limit: {bad}")
        _PROGRAM_CACHE[key] = nc
    return _PROGRAM_CACHE[key]


def _np32(a):
    return np.ascontiguousarray(np.asarray(a, dtype=np.float32))


def _mm_np_dtype():
    if MM_DTYPE == "bf16":
        import ml_dtypes

        return ml_dtypes.bfloat16
    return np.float32


def _pack_bT(a, Bc, K, mmdt):
    """[Bc, K] activations -> [128, KO*Bc] with per-partition layout
    [chunk][ko][b_local] (chunk-major, matching the per-chunk DMAs)."""
    KO = K // P
    parts, lo = [], 0
    for w in _bchunks(Bc):
        blk = a[lo:lo + w].reshape(w, KO, P).transpose(2, 1, 0)  # [ki, ko, b]
        parts.append(blk.reshape(P, KO * w))
        lo += w
    return np.ascontiguousarray(np.concatenate(parts, axis=1).astype(mmdt))


def _pack_w(w, mmdt):
    """[K, H] weight -> [128, KO*H] with per-partition layout [ko, h]."""
    K, H = w.shape
    out = w.reshape(K // P, P, H).transpose(1, 0, 2)
    return np.ascontiguousarray(out.reshape(P, -1).astype(mmdt))


def _prepare(x, att_score, hidden, W_u, U_u, b_u, W_r, U_r, b_r, W_h, U_h, b_h):
    x = _np32(x)
    att_score = _np32(att_score)
    hidden = _np32(hidden)
    B, D = x.shape
    H = hidden.shape[1]
    assert B % (NCORES * P) == 0 and D % P == 0 and H % P == 0
    Bc = B // NCORES
    mmdt = _mm_np_dtype()

    weights = {
        "wu": _np32(W_u), "wr": _np32(W_r), "wh": _np32(W_h),
        "uu": _np32(U_u), "ur": _np32(U_r), "uh": _np32(U_h),
    }
    biases = [_np32(b_u), _np32(b_r), _np32(b_h)]
    with_bias = any(np.any(b) for b in biases)
    packed_w = {k: _pack_w(v, mmdt) for k, v in weights.items()}

    in_maps = []
    for c in range(NCORES):
        sl = slice(c * Bc, (c + 1) * Bc)
        xs, hs, at = x[sl], hidden[sl], att_score[sl]
        m = {
            "xT": _pack_bT(xs, Bc, D, mmdt),
            "hT": _pack_bT(hs, Bc, H, mmdt),
            "hN": np.ascontiguousarray(hs),
            "att": np.ascontiguousarray(at.reshape(Bc // P, P).T),
        }
        m.update(packed_w)
        if with_bias:
            m["bub"] = np.ascontiguousarray(np.broadcast_to(biases[0], (P, H)))
            m["brb"] = np.ascontiguousarray(np.broadcast_to(biases[1], (P, H)))
            m["bhb"] = np.ascontiguousarray(np.broadcast_to(biases[2], (P, H)))
        in_maps.append(m)

    nc = _get_program(D, H, Bc, with_bias)
    return nc, in_maps


def _run(inputs, trace=False, **trace_kwargs):
    from concourse.bass_utils import run_bass_kernel_spmd

    nc, in_maps = _prepare(**inputs)
    res = run_bass_kernel_spmd(nc, in_maps, list(range(NCORES)), trace=trace,
                               **trace_kwargs)
    out = np.concatenate([res.results[i]["out"] for i in range(NCORES)], axis=0)
    return out, res


def kernel(**inputs):
    out, _ = _run(inputs, trace=False)
    return out


# revision 17
# speedup vs baseline: 1.1851x; 1.0171x over previous
"""AUGRU cell (attention-gated GRU update) on 8 Trainium2 NeuronCores.

Data-parallel: the batch dim (16384) of x / att_score / hidden is sharded
across 8 cores (2048 rows each); the six 512x512 weight matrices are
replicated.

Per-core dataflow (per 128-row batch tile, 16 tiles):
  zr = x @ W_r + h @ U_r          (PSUM accum, 8 matmuls)
  hu = h @ U_h ; xh = x @ W_h
  zu = x @ W_u + h @ U_u          (update gate last: shortest tail)
  r = sigmoid(zr); g = tanh(r * hu + xh); d = g - h
  u2 = att * sigmoid(zu)
  out = h + u2 * d                (== (1-u2)*h + u2*g)

Group order zr, hu, xh, zu means everything except the short
sigmoid(zu) -> u2 -> u2*d -> +h chain completes while the zu matmuls
still run; the last tile runs that chain in two H-halves to pipeline
ACT/DVE and cut the kernel tail.

Matmuls in bf16 (rel err ~2.4e-3 vs the 2e-2 gate). All matmul operands
are HOST-PREPACKED into DRAM buffers whose per-partition bytes are
contiguous in exactly the SBUF-resident layout, so every load is one
direct DMA (128 descriptors x 2-4KB) - no staging copies, no DVE casts.
DMAs are spread across the sync/scalar/gpsimd/vector engine queues so
weight and batch-chunk loads issue in parallel right after the framework
preamble; junk bf16 ldweights keep the PE busy (HAM warm) while the
first weights land. Each PSUM bank keeps a single releasing engine
(zu/zr: ACT sigmoid; hu/xh: DVE) so per-Matmult sync waits stay <=1;
stragglers are legalized by _split_multi_waits.
"""

import os
import sys

if "/opt/trn_rl_repo" not in sys.path:
    sys.path.insert(0, "/opt/trn_rl_repo")

import numpy as np

NCORES = 8
P = 128
MM_DTYPE = os.environ.get("MM_DTYPE", "bf16")  # "bf16" or "f32r"


def _bchunks(Bc):
    """Batch-chunk widths for the x/h loads: small early chunks so the
    first tiles' operands land fast, then wide chunks for bandwidth."""
    ws, rem = [], Bc
    for w in (P, P, 2 * P, 4 * P):
        if rem <= 0:
            break
        w = min(w, rem)
        ws.append(w)
        rem -= w
    while rem > 0:
        w = min(4 * P, rem)
        ws.append(w)
        rem -= w
    return ws

_PROGRAM_CACHE = {}


def _split_multi_waits(nc):
    """walrus codegen accepts at most ONE sync wait per instruction (the
    TPB EVENTS struct has a single wait slot and setupSyncWait refuses to
    spill).  Tile's add_semaphores can emit several waits on one
    instruction; hoist all but the last into same-engine no-ops inserted
    immediately before it.  The engine executes the no-ops (each blocking
    on one semaphore) then the instruction - identical semantics.

    Matmult/Ldweights get ALL waits hoisted: a wait carried on a PE
    instruction breaks the fill/drain overlap with the previous matmul
    (~210ns per occurrence, once per tile); a NoOp carrying the wait
    dispatches while the previous matmul still streams, so the pipeline
    stays full."""
    import concourse.mybir as mybir

    for fn in nc.m.functions:
        for blk in fn.blocks:
            insts = blk.instructions
            i = 0
            while i < len(insts):
                inst = insts[i]
                si = inst.sync_info
                nhoist = 0
                if si is not None and si.on_wait:
                    if type(inst).__name__ in ("InstMatmult", "InstLdweights"):
                        nhoist = len(si.on_wait)
                    elif len(si.on_wait) > 1:
                        nhoist = len(si.on_wait) - 1
                if nhoist:
                    waits = list(si.on_wait)
                    inst.sync_info = mybir.SyncInfo(
                        on_wait=waits[nhoist:], on_update=list(si.on_update)
                    )
                    for j, w in enumerate(waits[:nhoist]):
                        nop = mybir.InstNoOp(
                            name=nc.get_next_instruction_name(),
                            sync_info=mybir.SyncInfo(on_wait=[w], on_update=[]),
                            bass_nofuse=True,
                            engine=inst.engine,
                        )
                        nc.register_instruction(nop)
                        insts.insert(i + j, nop)
                    i += nhoist
                i += 1


def _build_program(D, H, Bc, with_bias, mm_dtype=None):
    import concourse.bass as bass
    import concourse.mybir as mybir
    import concourse.tile as tile
    from concourse.alu_op_type import AluOpType

    f32 = mybir.dt.float32
    bf16_mode = (mm_dtype or MM_DTYPE) == "bf16"
    mm_dt = mybir.dt.bfloat16 if bf16_mode else mybir.dt.float32r
    Sig = mybir.ActivationFunctionType.Sigmoid
    Tanh = mybir.ActivationFunctionType.Tanh

    KD = D // P  # K chunks for x-side matmuls
    KH = H // P  # K chunks for h-side matmuls
    TILES = Bc // P

    nc = bass.Bass()
    # Host-prepacked DRAM layouts: per-partition bytes contiguous, matching
    # the SBUF-resident tiles exactly (one fat descriptor per partition).
    xT_p = nc.declare_dram_parameter("xT", [P, KD * Bc], mm_dt, isOutput=False)
    hT_p = nc.declare_dram_parameter("hT", [P, KH * Bc], mm_dt, isOutput=False)
    hN_p = nc.declare_dram_parameter("hN", [Bc, H], mybir.dt.bfloat16, isOutput=False)
    att_p = nc.declare_dram_parameter("att", [P, TILES], f32, isOutput=False)
    wnames = ("wu", "wr", "wh", "uu", "ur", "uh")
    w_p = {n: nc.declare_dram_parameter(n, [P, (KD if n[0] == "w" else KH) * H],
                                        mm_dt, isOutput=False) for n in wnames}
    if with_bias:
        b_p = {n: nc.declare_dram_parameter(n, [P, H], f32, isOutput=False)
               for n in ("bub", "brb", "bhb")}
    out_p = nc.declare_dram_parameter("out", [Bc, H], f32, isOutput=True)

    wview = {n: w_p[n][:].rearrange("ki (ko h) -> ki ko h",
                                    ko=KD if n[0] == "w" else KH) for n in wnames}

    CH_W = _bchunks(Bc)

    with tile.TileContext(nc) as tc:
        with (
            tc.tile_pool(name="w", bufs=1) as wpool,
            tc.tile_pool(name="dat", bufs=4) as dpool,
            tc.tile_pool(name="ep", bufs=3) as epool,
            tc.tile_pool(name="ps", bufs=2, space="PSUM") as ppool,
        ):
            w_sb = {n: wpool.tile([P, KD if n[0] == "w" else KH, H], mm_dt,
                                  tag=n, name=f"w_{n}") for n in wnames}
            xT_sb = wpool.tile([P, KD, Bc], mm_dt, tag="xT")
            hT_sb = wpool.tile([P, KH, Bc], mm_dt, tag="hT")
            att_sb = wpool.tile([P, TILES], f32, tag="att")

            # PE warm-up: the HAM clock gate needs ~3.4us of sustained PE
            # activity before it lifts the array clock to 2.4 GHz. Junk
            # bf16 weight loads keep the PE busy while the first DMAs
            # land, so the real matmuls start warm. memset on gpsimd so
            # the LDWs are not gated behind any DMA-issuing engine.
            warm = wpool.tile([P, P], mybir.dt.bfloat16, tag="warm")
            nc.gpsimd.memset(warm, 0.0)
            for _ in range(20):
                nc.tensor.ldweights(warm)

            # Direct DMAs, spread across the three HWDGE queues (sync,
            # scalar, gpsimd), in consumption order. The first two tiles
            # run their zr/hu groups before any xh/zu (see below), so the
            # early need order is xc0, wr, hc0, ur, uh, then wh, wu, uu.
            #   sync:   x chunk0, wr halves, wu, att   then out stores
            #   scalar: ur halves, wh   then per-tile hN loads + ACTs
            #   gpsimd: h chunk0, uh halves, x/h chunks 1.., uu
            def chunk_dma(eng, sb, view, c):
                lo = sum(CH_W[:c])
                KO = sb.shape[1]
                src = view[:, KO * lo:KO * (lo + CH_W[c])].rearrange(
                    "ki (ko b) -> ki ko b", ko=KO)
                eng.dma_start(sb[:, :, lo:lo + CH_W[c]], src)

            def half_dma(eng, n, half):
                KO = w_sb[n].shape[1]
                sl = slice(0, KO // 2) if half == 0 else slice(KO // 2, KO)
                eng.dma_start(w_sb[n][:, sl], wview[n][:, sl])

            nc.sync.dma_start(xT_sb[:, :, 0:CH_W[0]],
                              xT_p[:, 0:KD * CH_W[0]].rearrange(
                                  "ki (ko b) -> ki ko b", ko=KD))
            nc.gpsimd.dma_start(hT_sb[:, :, 0:CH_W[0]],
                                hT_p[:, 0:KH * CH_W[0]].rearrange(
                                    "ki (ko b) -> ki ko b", ko=KH))
            half_dma(nc.sync, "wr", 0)
            half_dma(nc.scalar, "ur", 0)
            half_dma(nc.sync, "wr", 1)
            half_dma(nc.scalar, "ur", 1)
            half_dma(nc.gpsimd, "uh", 0)
            half_dma(nc.gpsimd, "uh", 1)
            nc.scalar.dma_start(w_sb["wh"], wview["wh"])
            nc.sync.dma_start(w_sb["wu"], wview["wu"])
            nc.sync.dma_start(att_sb, att_p[:])
            for c in range(1, len(CH_W)):
                chunk_dma(nc.gpsimd, xT_sb, xT_p[:], c)
                chunk_dma(nc.gpsimd, hT_sb, hT_p[:], c)
                if c == 1:
                    nc.gpsimd.dma_start(w_sb["uu"], wview["uu"])
            if with_bias:
                b_sb = {}
                for n in ("bub", "brb", "bhb"):
                    t = wpool.tile([P, H], f32, tag=n)
                    nc.scalar.dma_start(t, b_p[n][:])
                    b_sb[n] = t

            bf16 = mybir.dt.bfloat16
            psum = {}

            def mm_groups(t, names):
                bsl = slice(t * P, (t + 1) * P)
                lx = [xT_sb[:, ki, bsl] for ki in range(KD)]
                lh = [hT_sb[:, ki, bsl] for ki in range(KH)]
                for name in names:
                    if name in ("zr", "zu"):
                        pt = ppool.tile([P, H], f32, tag=name)
                        wx, wh_ = ("wr", "ur") if name == "zr" else ("wu", "uu")
                        for ki in range(KD):
                            nc.tensor.matmul(pt, lx[ki], w_sb[wx][:, ki],
                                             start=ki == 0, stop=False)
                        for ki in range(KH):
                            nc.tensor.matmul(pt, lh[ki], w_sb[wh_][:, ki],
                                             start=False, stop=ki == KH - 1)
                    elif name == "hu":
                        pt = ppool.tile([P, H], f32, tag="hu")
                        for ki in range(KH):
                            nc.tensor.matmul(pt, lh[ki], w_sb["uh"][:, ki],
                                             start=ki == 0, stop=ki == KH - 1)
                    else:  # xh
                        pt = ppool.tile([P, H], f32, tag="xh")
                        for ki in range(KD):
                            nc.tensor.matmul(pt, lx[ki], w_sb["wh"][:, ki],
                                             start=ki == 0, stop=ki == KD - 1)
                    psum[name] = pt

            def epilogue(t, split):
                bsl = slice(t * P, (t + 1) * P)
                h_t = dpool.tile([P, H], bf16, tag="h")
                nc.scalar.dma_start(h_t, hN_p[bsl, :])
                p_zr, p_hu, p_xh, p_zu = (psum[n] for n in ("zr", "hu", "xh", "zu"))

                # PSUM releasing engines: zr/zu by ACT sigmoid, hu/xh by
                # DVE. Group order zr, hu, xh, zu means the candidate
                # chain (r, r*hu+xh, tanh, -h) completes while the zu
                # matmuls still run; only sig(zu) -> u2 -> u2*d -> +h
                # trails the last matmul. The last tile runs in two
                # H-halves so ACT/DVE pipeline and the store issues early.
                u = epool.tile([P, H], f32, tag="u")
                r = epool.tile([P, H], f32, tag="r")
                u2 = epool.tile([P, H], f32, tag="u2")
                g = epool.tile([P, H], f32, tag="g")
                d = epool.tile([P, H], f32, tag="d")
                o = epool.tile([P, H], f32, tag="o")
                if with_bias:
                    zus = epool.tile([P, H], f32, tag="zus")
                    zrs = epool.tile([P, H], f32, tag="zrs")
                att_c = att_sb[:, t:t + 1]

                halves = ((slice(0, H),) if not split
                          else (slice(0, H // 2), slice(H // 2, H)))
                if with_bias:
                    nc.vector.tensor_add(zrs, p_zr, b_sb["brb"])
                    nc.scalar.activation(r, zrs, Sig)
                else:
                    nc.scalar.activation(r, p_zr, Sig)
                for hs in halves:
                    nc.vector.tensor_mul(g[:, hs], r[:, hs], p_hu[:, hs])
                    nc.vector.tensor_add(g[:, hs], g[:, hs], p_xh[:, hs])
                    if with_bias:
                        nc.vector.tensor_add(g[:, hs], g[:, hs], b_sb["bhb"][:, hs])
                    nc.scalar.activation(g[:, hs], g[:, hs], Tanh)
                    nc.vector.tensor_sub(d[:, hs], g[:, hs], h_t[:, hs])
                for hs in halves:
                    if with_bias:
                        nc.vector.tensor_add(zus[:, hs], p_zu[:, hs], b_sb["bub"][:, hs])
                        nc.scalar.activation(u[:, hs], zus[:, hs], Sig)
                    else:
                        nc.scalar.activation(u[:, hs], p_zu[:, hs], Sig)
                    nc.vector.tensor_scalar_mul(u2[:, hs], u[:, hs], att_c)
                    nc.vector.tensor_mul(u2[:, hs], u2[:, hs], d[:, hs])
                    nc.vector.tensor_add(o[:, hs], u2[:, hs], h_t[:, hs])
                    nc.sync.dma_start(out_p[bsl, hs], o[:, hs])

            # Software-pipelined start: tiles 0/1 run their zr/hu groups
            # (needing only wr/ur/uh + the first x/h chunk) before any
            # xh/zu, so the PE starts as soon as the first weights land
            # and rides out the rest of the weight-arrival window.
            if TILES >= 2:
                mm_groups(0, ("zr", "hu"))
                zr0, hu0 = psum["zr"], psum["hu"]
                mm_groups(1, ("zr", "hu"))
                zr1, hu1 = psum["zr"], psum["hu"]
                psum.update(zr=zr0, hu=hu0)
                mm_groups(0, ("xh", "zu"))
                epilogue(0, split=False)
                psum.update(zr=zr1, hu=hu1)
                mm_groups(1, ("xh", "zu"))
                epilogue(1, split=False)
                start = 2
            else:
                start = 0
            for t in range(start, TILES):
                mm_groups(t, ("zr", "hu", "xh", "zu"))
                epilogue(t, split=t == TILES - 1)

    _split_multi_waits(nc)
    return nc


def check_waits(nc):
    """Matmults and Drains may carry at most 1 sync wait on walrus; other
    instruction classes tolerate more (walrus splits them itself)."""
    bad = []
    for fn in nc.m.functions:
        for blk in fn.blocks:
            for inst in blk.instructions:
                si = inst.sync_info
                nw = len(si.on_wait) if si else 0
                kind = type(inst).__name__
                if nw > 1:
                    bad.append((inst.name, kind, nw))
    return bad


def _get_program(D, H, Bc, with_bias):
    key = (D, H, Bc, with_bias, MM_DTYPE)
    if key not in _PROGRAM_CACHE:
        nc = _build_program(D, H, Bc, with_bias)
        bad = check_waits(nc)
        if bad:
            raise RuntimeError(f"instructions over the sync-wait limit: {bad}")
        _PROGRAM_CACHE[key] = nc
    return _PROGRAM_CACHE[key]


def _np32(a):
    return np.ascontiguousarray(np.asarray(a, dtype=np.float32))


def _bf16():
    import ml_dtypes

    return ml_dtypes.bfloat16


def _mm_np_dtype():
    return _bf16() if MM_DTYPE == "bf16" else np.float32


def _pack_bT(a, Bc, K, mmdt):
    """[Bc, K] activations -> [128, KO*Bc] with per-partition layout
    [chunk][ko][b_local] (chunk-major, matching the per-chunk DMAs)."""
    KO = K // P
    parts, lo = [], 0
    for w in _bchunks(Bc):
        blk = a[lo:lo + w].reshape(w, KO, P).transpose(2, 1, 0)  # [ki, ko, b]
        parts.append(blk.reshape(P, KO * w))
        lo += w
    return np.ascontiguousarray(np.concatenate(parts, axis=1).astype(mmdt))


def _pack_w(w, mmdt):
    """[K, H] weight -> [128, KO*H] with per-partition layout [ko, h]."""
    K, H = w.shape
    out = w.reshape(K // P, P, H).transpose(1, 0, 2)
    return np.ascontiguousarray(out.reshape(P, -1).astype(mmdt))


def _prepare(x, att_score, hidden, W_u, U_u, b_u, W_r, U_r, b_r, W_h, U_h, b_h):
    x = _np32(x)
    att_score = _np32(att_score)
    hidden = _np32(hidden)
    B, D = x.shape
    H = hidden.shape[1]
    assert B % (NCORES * P) == 0 and D % P == 0 and H % P == 0
    Bc = B // NCORES
    mmdt = _mm_np_dtype()

    weights = {
        "wu": _np32(W_u), "wr": _np32(W_r), "wh": _np32(W_h),
        "uu": _np32(U_u), "ur": _np32(U_r), "uh": _np32(U_h),
    }
    biases = [_np32(b_u), _np32(b_r), _np32(b_h)]
    with_bias = any(np.any(b) for b in biases)
    packed_w = {k: _pack_w(v, mmdt) for k, v in weights.items()}

    in_maps = []
    for c in range(NCORES):
        sl = slice(c * Bc, (c + 1) * Bc)
        xs, hs, at = x[sl], hidden[sl], att_score[sl]
        m = {
            "xT": _pack_bT(xs, Bc, D, mmdt),
            "hT": _pack_bT(hs, Bc, H, mmdt),
            "hN": np.ascontiguousarray(hs.astype(_bf16())),
            "att": np.ascontiguousarray(at.reshape(Bc // P, P).T),
        }
        m.update(packed_w)
        if with_bias:
            m["bub"] = np.ascontiguousarray(np.broadcast_to(biases[0], (P, H)))
            m["brb"] = np.ascontiguousarray(np.broadcast_to(biases[1], (P, H)))
            m["bhb"] = np.ascontiguousarray(np.broadcast_to(biases[2], (P, H)))
        in_maps.append(m)

    nc = _get_program(D, H, Bc, with_bias)
    return nc, in_maps


def _run(inputs, trace=False, **trace_kwargs):
    from concourse.bass_utils import run_bass_kernel_spmd

    nc, in_maps = _prepare(**inputs)
    res = run_bass_kernel_spmd(nc, in_maps, list(range(NCORES)), trace=trace,
                               **trace_kwargs)
    out = np.concatenate([res.results[i]["out"] for i in range(NCORES)], axis=0)
    return out, res


def kernel(**inputs):
    out, _ = _run(inputs, trace=False)
    return out


# revision 21
# speedup vs baseline: 1.1890x; 1.0033x over previous
"""AUGRU cell (attention-gated GRU update) on 8 Trainium2 NeuronCores.

Data-parallel: the batch dim (16384) of x / att_score / hidden is sharded
across 8 cores (2048 rows each); the six 512x512 weight matrices are
replicated.

Per-core dataflow (per 128-row batch tile, 16 tiles):
  zr = x @ W_r + h @ U_r          (PSUM accum, 8 matmuls)
  hu = h @ U_h ; xh = x @ W_h
  zu = x @ W_u + h @ U_u          (update gate last: shortest tail)
  r = sigmoid(zr); g = tanh(r * hu + xh); d = g - h
  u2 = att * sigmoid(zu)
  out = h + u2 * d                (== (1-u2)*h + u2*g)

Group order zr, hu, xh, zu means everything except the short
sigmoid(zu) -> u2 -> u2*d -> +h chain completes while the zu matmuls
still run; the last tile runs that chain in two H-halves to pipeline
ACT/DVE and cut the kernel tail.

Matmuls in bf16 (rel err ~2.4e-3 vs the 2e-2 gate). All matmul operands
are HOST-PREPACKED into DRAM buffers whose per-partition bytes are
contiguous in exactly the SBUF-resident layout, so every load is one
direct DMA (128 descriptors x 2-4KB) - no staging copies, no DVE casts.
DMAs are spread across the sync/scalar/gpsimd/vector engine queues so
weight and batch-chunk loads issue in parallel right after the framework
preamble; junk bf16 ldweights keep the PE busy (HAM warm) while the
first weights land. Each PSUM bank keeps a single releasing engine
(zu/zr: ACT sigmoid; hu/xh: DVE) so per-Matmult sync waits stay <=1;
stragglers are legalized by _split_multi_waits.
"""

import os
import sys

if "/opt/trn_rl_repo" not in sys.path:
    sys.path.insert(0, "/opt/trn_rl_repo")

import numpy as np

NCORES = 8
P = 128
MM_DTYPE = os.environ.get("MM_DTYPE", "bf16")  # "bf16" or "f32r"


def _bchunks(Bc):
    """Batch-chunk widths for the x/h loads: a small first chunk covering
    the software-pipelined first two tiles, then wide chunks."""
    ws, rem = [], Bc
    for w in (2 * P, 2 * P):
        if rem <= 0:
            break
        w = min(w, rem)
        ws.append(w)
        rem -= w
    while rem > 0:
        w = min(4 * P, rem)
        ws.append(w)
        rem -= w
    return ws

_PROGRAM_CACHE = {}


def _split_multi_waits(nc):
    """walrus codegen accepts at most ONE sync wait per instruction (the
    TPB EVENTS struct has a single wait slot and setupSyncWait refuses to
    spill).  Tile's add_semaphores can emit several waits on one
    instruction; hoist all but the last into same-engine no-ops inserted
    immediately before it.  The engine executes the no-ops (each blocking
    on one semaphore) then the instruction - identical semantics.

    Matmult/Ldweights get ALL waits hoisted: a wait carried on a PE
    instruction breaks the fill/drain overlap with the previous matmul
    (~210ns per occurrence, once per tile); a NoOp carrying the wait
    dispatches while the previous matmul still streams, so the pipeline
    stays full."""
    import concourse.mybir as mybir

    for fn in nc.m.functions:
        for blk in fn.blocks:
            insts = blk.instructions
            i = 0
            while i < len(insts):
                inst = insts[i]
                si = inst.sync_info
                nhoist = 0
                if si is not None and si.on_wait:
                    if type(inst).__name__ in ("InstMatmult", "InstLdweights"):
                        nhoist = len(si.on_wait)
                    elif len(si.on_wait) > 1:
                        nhoist = len(si.on_wait) - 1
                if nhoist:
                    waits = list(si.on_wait)
                    inst.sync_info = mybir.SyncInfo(
                        on_wait=waits[nhoist:], on_update=list(si.on_update)
                    )
                    for j, w in enumerate(waits[:nhoist]):
                        nop = mybir.InstNoOp(
                            name=nc.get_next_instruction_name(),
                            sync_info=mybir.SyncInfo(on_wait=[w], on_update=[]),
                            bass_nofuse=True,
                            engine=inst.engine,
                        )
                        nc.register_instruction(nop)
                        insts.insert(i + j, nop)
                    i += nhoist
                i += 1


def _build_program(D, H, Bc, with_bias, mm_dtype=None):
    import concourse.bass as bass
    import concourse.mybir as mybir
    import concourse.tile as tile
    from concourse.alu_op_type import AluOpType

    f32 = mybir.dt.float32
    bf16_mode = (mm_dtype or MM_DTYPE) == "bf16"
    mm_dt = mybir.dt.bfloat16 if bf16_mode else mybir.dt.float32r
    Sig = mybir.ActivationFunctionType.Sigmoid
    Tanh = mybir.ActivationFunctionType.Tanh

    KD = D // P  # K chunks for x-side matmuls
    KH = H // P  # K chunks for h-side matmuls
    TILES = Bc // P

    nc = bass.Bass()
    # Host-prepacked DRAM layouts: per-partition bytes contiguous, matching
    # the SBUF-resident tiles exactly (one fat descriptor per partition).
    xT_p = nc.declare_dram_parameter("xT", [P, KD * Bc], mm_dt, isOutput=False)
    hT_p = nc.declare_dram_parameter("hT", [P, KH * Bc], mm_dt, isOutput=False)
    hN_p = nc.declare_dram_parameter("hN", [Bc, H], mybir.dt.bfloat16, isOutput=False)
    att_p = nc.declare_dram_parameter("att", [P, TILES], f32, isOutput=False)
    wnames = ("wu", "wr", "wh", "uu", "ur", "uh")
    w_p = {n: nc.declare_dram_parameter(n, [P, (KD if n[0] == "w" else KH) * H],
                                        mm_dt, isOutput=False) for n in wnames}
    if with_bias:
        b_p = {n: nc.declare_dram_parameter(n, [P, H], f32, isOutput=False)
               for n in ("bub", "brb", "bhb")}
    out_p = nc.declare_dram_parameter("out", [Bc, H], f32, isOutput=True)

    wview = {n: w_p[n][:].rearrange("ki (ko h) -> ki ko h",
                                    ko=KD if n[0] == "w" else KH) for n in wnames}

    CH_W = _bchunks(Bc)

    with tile.TileContext(nc) as tc:
        with (
            tc.tile_pool(name="w", bufs=1) as wpool,
            tc.tile_pool(name="dat", bufs=4) as dpool,
            tc.tile_pool(name="ep", bufs=3) as epool,
            tc.tile_pool(name="ps", bufs=2, space="PSUM") as ppool,
        ):
            w_sb = {n: wpool.tile([P, KD if n[0] == "w" else KH, H], mm_dt,
                                  tag=n, name=f"w_{n}") for n in wnames}
            xT_sb = wpool.tile([P, KD, Bc], mm_dt, tag="xT")
            hT_sb = wpool.tile([P, KH, Bc], mm_dt, tag="hT")
            att_sb = wpool.tile([P, TILES], f32, tag="att")

            # PE warm-up: the HAM clock gate needs ~3.4us of sustained PE
            # activity before it lifts the array clock to 2.4 GHz. Junk
            # bf16 weight loads keep the PE busy while the first DMAs
            # land, so the real matmuls start warm. memset on gpsimd so
            # the LDWs are not gated behind any DMA-issuing engine.
            warm = wpool.tile([P, P], mybir.dt.bfloat16, tag="warm")
            nc.gpsimd.memset(warm, 0.0)
            for _ in range(20):
                nc.tensor.ldweights(warm)

            # Direct DMAs, spread across the three HWDGE queues (sync,
            # scalar, gpsimd), in consumption order. The first two tiles
            # run their zr/hu groups before any xh/zu (see below), so the
            # early need order is xc0, wr, hc0, ur, uh, then wh, wu, uu.
            #   sync:   x chunk0, wr halves, wu, att   then out stores
            #   scalar: ur halves, wh   then per-tile hN loads + ACTs
            #   gpsimd: h chunk0, uh halves, x/h chunks 1.., uu
            def chunk_dma(eng, sb, view, c):
                lo = sum(CH_W[:c])
                KO = sb.shape[1]
                src = view[:, KO * lo:KO * (lo + CH_W[c])].rearrange(
                    "ki (ko b) -> ki ko b", ko=KO)
                eng.dma_start(sb[:, :, lo:lo + CH_W[c]], src)

            def half_dma(eng, n, half):
                KO = w_sb[n].shape[1]
                sl = slice(0, KO // 2) if half == 0 else slice(KO // 2, KO)
                eng.dma_start(w_sb[n][:, sl], wview[n][:, sl])

            half_dma(nc.sync, "wr", 0)
            half_dma(nc.scalar, "ur", 0)
            chunk_dma(nc.gpsimd, xT_sb, xT_p[:], 0)
            half_dma(nc.sync, "wr", 1)
            half_dma(nc.scalar, "ur", 1)
            chunk_dma(nc.gpsimd, hT_sb, hT_p[:], 0)
            half_dma(nc.gpsimd, "uh", 0)
            half_dma(nc.gpsimd, "uh", 1)
            nc.scalar.dma_start(w_sb["wh"], wview["wh"])
            nc.sync.dma_start(w_sb["wu"], wview["wu"])
            nc.sync.dma_start(att_sb, att_p[:])
            nc.gpsimd.dma_start(w_sb["uu"], wview["uu"])
            for c in range(1, len(CH_W)):
                chunk_dma(nc.gpsimd, xT_sb, xT_p[:], c)
                chunk_dma(nc.gpsimd, hT_sb, hT_p[:], c)
            if with_bias:
                b_sb = {}
                for n in ("bub", "brb", "bhb"):
                    t = wpool.tile([P, H], f32, tag=n)
                    nc.scalar.dma_start(t, b_p[n][:])
                    b_sb[n] = t

            bf16 = mybir.dt.bfloat16
            psum = {}

            GROUP_W = {"zr": ("wr", "ur"), "zu": ("wu", "uu"),
                       "hu": (None, "uh"), "xh": ("wh", None)}

            def mm_piece(t, name, side, kis):
                """Emit the matmuls of group `name` for tile t restricted
                to `kis` of `side` ('x'/'h'). start/stop flags derive from
                the group's overall first/last matmul."""
                bsl = slice(t * P, (t + 1) * P)
                wx, wh_ = GROUP_W[name]
                pt = psum[name]
                first_side = "x" if wx else "h"
                last_side = "h" if wh_ else "x"
                for ki in kis:
                    if side == "x":
                        nc.tensor.matmul(pt, xT_sb[:, ki, bsl], w_sb[wx][:, ki],
                                         start=side == first_side and ki == 0,
                                         stop=side == last_side and ki == KD - 1)
                    else:
                        nc.tensor.matmul(pt, hT_sb[:, ki, bsl], w_sb[wh_][:, ki],
                                         start=side == first_side and ki == 0,
                                         stop=side == last_side and ki == KH - 1)

            def new_group(t, name):
                psum[name] = ppool.tile([P, H], f32, tag=name,
                                        name=f"p_{name}_{t}")

            def mm_groups(t, names):
                for name in names:
                    new_group(t, name)
                    wx, wh_ = GROUP_W[name]
                    if wx:
                        mm_piece(t, name, "x", range(KD))
                    if wh_:
                        mm_piece(t, name, "h", range(KH))

            def epilogue(t, saved=None):
                bsl = slice(t * P, (t + 1) * P)
                h_t = dpool.tile([P, H], bf16, tag="h")
                nc.scalar.dma_start(h_t, hN_p[bsl, :])
                ps = saved or psum
                p_zr, p_hu, p_xh, p_zu = (ps[n] for n in ("zr", "hu", "xh", "zu"))

                # PSUM releasing engines: zr/zu by ACT sigmoid, hu/xh by
                # DVE. Group order zr, hu, xh, zu means the candidate
                # chain (r, r*hu+xh, tanh, -h) completes while the zu
                # matmuls still run; only sig(zu) -> stt -> +h trails the
                # last matmul.
                u = epool.tile([P, H], f32, tag="u")
                r = epool.tile([P, H], f32, tag="r")
                g = epool.tile([P, H], f32, tag="g")
                d = epool.tile([P, H], f32, tag="d")
                o = epool.tile([P, H], f32, tag="o")
                if with_bias:
                    zus = epool.tile([P, H], f32, tag="zus")
                    zrs = epool.tile([P, H], f32, tag="zrs")
                att_c = att_sb[:, t:t + 1]

                if with_bias:
                    nc.vector.tensor_add(zrs, p_zr, b_sb["brb"])
                    nc.scalar.activation(r, zrs, Sig)
                else:
                    nc.scalar.activation(r, p_zr, Sig)
                nc.vector.tensor_mul(g, r, p_hu)       # r * (h @ U_h)
                nc.vector.tensor_add(g, g, p_xh)       # + x @ W_h
                if with_bias:
                    nc.vector.tensor_add(g, g, b_sb["bhb"])
                nc.scalar.activation(g, g, Tanh)       # hhat
                nc.vector.tensor_sub(d, g, h_t)        # hhat - h
                if with_bias:
                    nc.vector.tensor_add(zus, p_zu, b_sb["bub"])
                    nc.scalar.activation(u, zus, Sig)
                else:
                    nc.scalar.activation(u, p_zu, Sig)
                # m = (u * att) * d, fused on DVE
                nc.vector.scalar_tensor_tensor(d, u, att_c, d,
                                               AluOpType.mult, AluOpType.mult)
                nc.vector.tensor_add(o, d, h_t)        # h + u2*(hhat-h)
                nc.sync.dma_start(out_p[bsl, :], o)

            # Software-pipelined start: tiles 0/1 are emitted one
            # weight-piece at a time across both tiles, in the exact
            # order the weight DMAs land (wr halves, ur halves, uh
            # halves, wh, wu, uu). The PE starts on the first quarter
            # weight and consumes each piece as it arrives instead of
            # stalling on any one tile's full weight set.
            if TILES >= 2:
                saved = [{} for _ in range(2)]
                for t in (0, 1):
                    new_group(t, "zr")
                    saved[t]["zr"] = psum["zr"]
                hk = KD // 2
                for t in (0, 1):
                    psum["zr"] = saved[t]["zr"]
                    mm_piece(t, "zr", "x", range(hk))
                for t in (0, 1):
                    psum["zr"] = saved[t]["zr"]
                    mm_piece(t, "zr", "x", range(hk, KD))
                for t in (0, 1):
                    psum["zr"] = saved[t]["zr"]
                    mm_piece(t, "zr", "h", range(hk))
                for t in (0, 1):
                    psum["zr"] = saved[t]["zr"]
                    mm_piece(t, "zr", "h", range(hk, KH))
                for t in (0, 1):
                    new_group(t, "hu")
                    saved[t]["hu"] = psum["hu"]
                    mm_piece(t, "hu", "h", range(hk))
                for t in (0, 1):
                    psum["hu"] = saved[t]["hu"]
                    mm_piece(t, "hu", "h", range(hk, KH))
                for t in (0, 1):
                    new_group(t, "xh")
                    saved[t]["xh"] = psum["xh"]
                    mm_piece(t, "xh", "x", range(KD))
                for t in (0, 1):
                    new_group(t, "zu")
                    saved[t]["zu"] = psum["zu"]
                    mm_piece(t, "zu", "x", range(KD))
                for t in (0, 1):
                    psum["zu"] = saved[t]["zu"]
                    mm_piece(t, "zu", "h", range(KH))
                epilogue(0, saved[0])
                epilogue(1, saved[1])
                start = 2
            else:
                start = 0
            for t in range(start, TILES):
                mm_groups(t, ("zr", "hu", "xh", "zu"))
                epilogue(t)

    _split_multi_waits(nc)
    return nc


def check_waits(nc):
    """Matmults and Drains may carry at most 1 sync wait on walrus; other
    instruction classes tolerate more (walrus splits them itself)."""
    bad = []
    for fn in nc.m.functions:
        for blk in fn.blocks:
            for inst in blk.instructions:
                si = inst.sync_info
                nw = len(si.on_wait) if si else 0
                kind = type(inst).__name__
                if nw > 1:
                    bad.append((inst.name, kind, nw))
    return bad


def _get_program(D, H, Bc, with_bias):
    key = (D, H, Bc, with_bias, MM_DTYPE)
    if key not in _PROGRAM_CACHE:
        nc = _build_program(D, H, Bc, with_bias)
        bad = check_waits(nc)
        if bad:
            raise RuntimeError(f"instructions over the sync-wait limit: {bad}")
        _PROGRAM_CACHE[key] = nc
    return _PROGRAM_CACHE[key]


def _np32(a):
    return np.ascontiguousarray(np.asarray(a, dtype=np.float32))


def _bf16():
    import ml_dtypes

    return ml_dtypes.bfloat16


def _mm_np_dtype():
    return _bf16() if MM_DTYPE == "bf16" else np.float32


def _pack_bT(a, Bc, K, mmdt):
    """[Bc, K] activations -> [128, KO*Bc] with per-partition layout
    [chunk][ko][b_local] (chunk-major, matching the per-chunk DMAs)."""
    KO = K // P
    parts, lo = [], 0
    for w in _bchunks(Bc):
        blk = a[lo:lo + w].reshape(w, KO, P).transpose(2, 1, 0)  # [ki, ko, b]
        parts.append(blk.reshape(P, KO * w))
        lo += w
    return np.ascontiguousarray(np.concatenate(parts, axis=1).astype(mmdt))


def _pack_w(w, mmdt):
    """[K, H] weight -> [128, KO*H] with per-partition layout [ko, h]."""
    K, H = w.shape
    out = w.reshape(K // P, P, H).transpose(1, 0, 2)
    return np.ascontiguousarray(out.reshape(P, -1).astype(mmdt))


def _prepare(x, att_score, hidden, W_u, U_u, b_u, W_r, U_r, b_r, W_h, U_h, b_h):
    x = _np32(x)
    att_score = _np32(att_score)
    hidden = _np32(hidden)
    B, D = x.shape
    H = hidden.shape[1]
    assert B % (NCORES * P) == 0 and D % P == 0 and H % P == 0
    Bc = B // NCORES
    mmdt = _mm_np_dtype()

    weights = {
        "wu": _np32(W_u), "wr": _np32(W_r), "wh": _np32(W_h),
        "uu": _np32(U_u), "ur": _np32(U_r), "uh": _np32(U_h),
    }
    biases = [_np32(b_u), _np32(b_r), _np32(b_h)]
    with_bias = any(np.any(b) for b in biases)
    packed_w = {k: _pack_w(v, mmdt) for k, v in weights.items()}

    in_maps = []
    for c in range(NCORES):
        sl = slice(c * Bc, (c + 1) * Bc)
        xs, hs, at = x[sl], hidden[sl], att_score[sl]
        m = {
            "xT": _pack_bT(xs, Bc, D, mmdt),
            "hT": _pack_bT(hs, Bc, H, mmdt),
            "hN": np.ascontiguousarray(hs.astype(_bf16())),
            "att": np.ascontiguousarray(at.reshape(Bc // P, P).T),
        }
        m.update(packed_w)
        if with_bias:
            m["bub"] = np.ascontiguousarray(np.broadcast_to(biases[0], (P, H)))
            m["brb"] = np.ascontiguousarray(np.broadcast_to(biases[1], (P, H)))
            m["bhb"] = np.ascontiguousarray(np.broadcast_to(biases[2], (P, H)))
        in_maps.append(m)

    nc = _get_program(D, H, Bc, with_bias)
    return nc, in_maps


def _run(inputs, trace=False, **trace_kwargs):
    from concourse.bass_utils import run_bass_kernel_spmd

    nc, in_maps = _prepare(**inputs)
    res = run_bass_kernel_spmd(nc, in_maps, list(range(NCORES)), trace=trace,
                               **trace_kwargs)
    out = np.concatenate([res.results[i]["out"] for i in range(NCORES)], axis=0)
    return out, res


def kernel(**inputs):
    out, _ = _run(inputs, trace=False)
    return out
